# revision 1
# baseline (speedup 1.0000x reference)
"""Trainium2 Bass kernel for BaseCausalWanSelfAttention (local+sink sparse attention
with interleaved rotary), SPMD across 8 NeuronCores.

Sharding: the 24 (batch, head) pairs are split 3-per-core across 8 cores; each
core runs full local+sink attention for its pairs independently (no collectives).
"""
import sys

sys.path.insert(0, "/opt/trn_rl_repo")

import numpy as np

import concourse.bacc as bacc
import concourse.tile as tile
import concourse.mybir as mybir

dt = mybir.dt

# Problem config (hardcoded per contest contract)
B, S, H, D = 2, 3072, 12, 128
LOCAL_WINDOW = 1560
SINK = 128
N_CORES = 8
PER_CORE = (B * H) // N_CORES  # 3
QB = 512  # query block (columns of transposed scores)
NQC = QB // 128  # 128-query chunks per block
SCALE = 1.0 / float(np.sqrt(D))


def _window_partial_deltas(w):
    """k-tile offsets (qi - kj) where the local-window edge cuts through the
    128x128 tile; maps delta -> threshold T with allowed iff (c - p) < T."""
    out = {}
    for d in range((w - 127 + 127) // 128, (w + 127) // 128 + 1):
        t = w - 128 * d
        if -127 <= t <= 127:
            out[d] = t
    return out


def chunk_kinds(qb, kj, w=LOCAL_WINDOW, nqc=NQC):
    """Per 128-query chunk classification of k-tile kj for query block qb.
    Returns list of (t, kind) with kind in {"full", "diag", ("win", delta)} for
    valid chunks only. SINK==128 assumed (k-tile 0 fully attendable)."""
    partial = _window_partial_deltas(w)
    max_delta = max(partial) if partial else (w - 1) // 128
    kinds = []
    for t in range(nqc):
        qi = nqc * qb + t
        if kj == 0:
            kinds.append((t, "diag" if qi == 0 else "full"))
            continue
        delta = qi - kj
        if delta < 0 or delta > max_delta:
            continue
        if delta == 0:
            kinds.append((t, "diag"))
        elif delta in partial:
            kinds.append((t, ("win", delta)))
        else:
            kinds.append((t, "full"))
    return kinds


def kj_list(qb, s=S, w=LOCAL_WINDOW, nqc=NQC):
    partial = _window_partial_deltas(w)
    max_delta = max(partial) if partial else (w - 1) // 128
    n_ktiles = s // 128
    hi = min(nqc * qb + nqc - 1, n_ktiles - 1)
    lo = max(1, nqc * qb - max_delta)
    return [0] + [kj for kj in range(lo, hi + 1)]


def build_nc(s=S, per_core=PER_CORE, w=LOCAL_WINDOW):
    """Build the SPMD single-core program (identical on all cores)."""
    nqb = s // QB
    partial = _window_partial_deltas(w)

    nc = bacc.Bacc("TRN2", target_bir_lowering=False, debug=False)

    qT = nc.declare_dram_parameter("qT", [per_core, 128, s], dt.float32r, isOutput=False)
    kT = nc.declare_dram_parameter("kT", [per_core, 128, s], dt.float32r, isOutput=False)
    v = nc.declare_dram_parameter("v", [per_core, s, 128], dt.float32r, isOutput=False)
    cexpT = nc.declare_dram_parameter("cexpT", [128, s], dt.float32r, isOutput=False)
    ssigT = nc.declare_dram_parameter("ssigT", [128, s], dt.float32r, isOutput=False)
    pswap = nc.declare_dram_parameter("pswap", [128, 128], dt.float32r, isOutput=False)
    ident = nc.declare_dram_parameter("ident", [128, 128], dt.float32, isOutput=False)
    ones = nc.declare_dram_parameter("ones", [128, 128], dt.float32r, isOutput=False)
    maskD = nc.declare_dram_parameter("maskD", [128, 128], dt.float32r, isOutput=False)
    wmask_names = {}
    for delta in sorted(partial):
        nm = f"maskW{delta}"
        wmask_names[delta] = nc.declare_dram_parameter(
            nm, [128, 128], dt.float32r, isOutput=False
        )
    out = nc.declare_dram_parameter("out", [per_core, s, 128], dt.float32, isOutput=True)

    with tile.TileContext(nc) as tc:
        with (
            tc.tile_pool(name="const", bufs=1) as cpool,
            tc.tile_pool(name="big", bufs=2) as bigpool,
            tc.tile_pool(name="probs", bufs=7) as ppool,
            tc.tile_pool(name="tail", bufs=2) as tpool,
            tc.tile_pool(name="ps_sc", bufs=5, space="PSUM") as ps_sc,
            tc.tile_pool(name="ps_out", bufs=2, space="PSUM") as ps_out,
            tc.tile_pool(name="ps_den", bufs=1, space="PSUM") as ps_den,
        ):
            # constants
            cexp_sb = cpool.tile([128, s], dt.float32r, tag="cexp")
            ssig_sb = cpool.tile([128, s], dt.float32r, tag="ssig")
            pswap_sb = cpool.tile([128, 128], dt.float32r, tag="pswap")
            nc.sync.dma_start(out=pswap_sb[:], in_=pswap[:])
            nc.sync.dma_start(out=cexp_sb[:, 0:1024], in_=cexpT[:, 0:1024])
            nc.sync.dma_start(out=ssig_sb[:, 0:1024], in_=ssigT[:, 0:1024])
            ident_sb = cpool.tile([128, 128], dt.float32, tag="ident")
            ones_sb = cpool.tile([128, 128], dt.float32r, tag="ones")
            maskD_sb = cpool.tile([128, 128], dt.float32r, tag="maskD")
            wdeltas = sorted(wmask_names)
            wmask_sb = {
                delta: cpool.tile(
                    [128, 128], dt.float32r, tag=f"maskW{delta}", name=f"mW{delta}"
                )
                for delta in wdeltas
            }
            wpair_sb = None
            if len(wdeltas) == 2 and wdeltas[1] == wdeltas[0] + 1:
                wpair_sb = cpool.tile([128, 256], dt.float32r, tag="maskWpair")

            def load_consts_rest():
                nc.sync.dma_start(out=ident_sb[:], in_=ident[:])
                nc.sync.dma_start(out=ones_sb[:], in_=ones[:])
                nc.sync.dma_start(out=maskD_sb[:], in_=maskD[:])
                for delta, m in wmask_sb.items():
                    nc.sync.dma_start(out=m[:], in_=wmask_names[delta][:])
                if wpair_sb is not None:
                    nc.sync.dma_start(
                        out=wpair_sb[:, 0:128], in_=wmask_names[wdeltas[0]][:]
                    )
                    nc.sync.dma_start(
                        out=wpair_sb[:, 128:256], in_=wmask_names[wdeltas[1]][:]
                    )
                for c2 in range(1, s // 1024):
                    sl2 = slice(c2 * 1024, (c2 + 1) * 1024)
                    nc.sync.dma_start(out=cexp_sb[:, sl2], in_=cexpT[:, sl2])
                    nc.sync.dma_start(out=ssig_sb[:, sl2], in_=ssigT[:, sl2])

            def load(u):
                qraw = bigpool.tile([128, s], dt.float32r, tag="qraw")
                kraw = bigpool.tile([128, s], dt.float32r, tag="kraw")
                v_sb = bigpool.tile([128, s], dt.float32r, tag="v")
                for c2 in range(s // 1024):
                    sl2 = slice(c2 * 1024, (c2 + 1) * 1024)
                    nc.sync.dma_start(out=qraw[:, sl2], in_=qT[u][:, sl2])
                    nc.sync.dma_start(out=kraw[:, sl2], in_=kT[u][:, sl2])
                nc.sync.dma_start(
                    out=v_sb[:].rearrange("p (n d) -> p n d", d=128),
                    in_=v[u].rearrange("(n p) d -> p n d", p=128),
                )
                rq = bigpool.tile([128, s], dt.float32r, tag="rq")
                rk = bigpool.tile([128, s], dt.float32r, tag="rk")
                return qraw, kraw, v_sb, rq, rk

            def rotary(tiles, lo, hi):
                """Rotary for columns [lo,hi) of q and k; 512-col DVE chunks."""
                qraw, kraw, v_sb, rq, rk = tiles
                for raw, r in ((qraw, rq), (kraw, rk)):
                    step = 1024 if (hi - lo) % 1024 == 0 else 512
                    for c in range(lo // step, hi // step):
                        sl = slice(c * step, (c + 1) * step)
                        sws = []
                        for h2 in range(step // 512):
                            ssl = slice(c * step + h2 * 512, c * step + (h2 + 1) * 512)
                            sw = ps_sc.tile([128, 512], dt.float32, tag="sc")
                            nc.tensor.matmul(
                                sw[:], pswap_sb[:], raw[:, ssl], start=True, stop=True
                            )
                            sws.append((ssl, sw))
                        # r = raw * cexp
                        nc.vector.tensor_mul(r[:, sl], raw[:, sl], cexp_sb[:, sl])
                        # raw <- swap(raw) * ssig  (psum src; raw reused as scratch)
                        for ssl, sw in sws:
                            nc.vector.tensor_mul(
                                raw[:, ssl], sw[:].bitcast(dt.float32r), ssig_sb[:, ssl]
                            )
                        # r += scratch
                        nc.vector.tensor_add(r[:, sl], r[:, sl], raw[:, sl])

            def emit_masks(probs, kinds):
                mk = [k for k in kinds if k[1] != "full"]
                j = 0
                while j < len(mk):
                    t, kind = mk[j]
                    if (
                        wpair_sb is not None
                        and j + 1 < len(mk)
                        and kind != "diag"
                        and mk[j + 1][1] != "diag"
                        and mk[j + 1][0] == t + 1
                        and kind[1] == wdeltas[0]
                    ):
                        tsl = slice(t * 128, (t + 2) * 128)
                        nc.vector.tensor_mul(probs[:, tsl], probs[:, tsl], wpair_sb[:])
                        j += 2
                        continue
                    m = maskD_sb if kind == "diag" else wmask_sb[kind[1]]
                    tsl = slice(t * 128, (t + 1) * 128)
                    nc.vector.tensor_mul(probs[:, tsl], probs[:, tsl], m[:])
                    j += 1

            def qb_order(qb):
                kjs = kj_list(qb, s=s, w=w)
                tiles = []
                for kj in kjs:
                    kinds = chunk_kinds(qb, kj, w=w)
                    assert kinds, (qb, kj)
                    tiles.append((kj, kinds, kinds[0][0], kinds[-1][0] + 1))
                fulls = [x for x in tiles if x[3] - x[2] == NQC]
                parts = [x for x in tiles if x[3] - x[2] != NQC]
                assert fulls[0][0] == 0
                order = [fulls[0]]
                rest_f = fulls[1:]
                rest_p = list(parts)
                stride = (
                    max(1, len(rest_f) // (len(rest_p) + 1))
                    if rest_p
                    else len(rest_f) or 1
                )
                while rest_f or rest_p:
                    order.extend(rest_f[:stride])
                    rest_f = rest_f[stride:]
                    if rest_p:
                        order.append(rest_p.pop(0))
                return order

            WAVE = 3
            state = {"pv": [], "tail": None}

            def flush_pv():
                if state["pv"]:
                    state["pv"].pop(0)()

            def flush_all():
                while state["pv"]:
                    flush_pv()

            def attention_qb(u, rq, rk, v_sb, qb):
                order = qb_order(qb)
                n_tiles = len(order)
                qbctx = {}

                def get_psums():
                    if "outT" not in qbctx:
                        outT_ps = ps_out.tile([128, QB], dt.float32, tag="outT")
                        den_ps = ps_den.tile([128, QB], dt.float32, tag="den")
                        qbctx["outT"] = outT_ps
                        qbctx["den"] = den_ps
                    return qbctx["outT"], qbctx["den"]

                for w0 in range(0, n_tiles, WAVE):
                    wave = order[w0 : w0 + WAVE]
                    wprobs = []
                    for kj, kinds, t0, t1 in wave:
                        csl = slice(qb * QB + t0 * 128, qb * QB + t1 * 128)
                        psl = slice(t0 * 128, t1 * 128)
                        ksl = slice(kj * 128, (kj + 1) * 128)
                        sc = ps_sc.tile([128, QB], dt.float32, tag="sc")
                        nc.tensor.matmul(
                            sc[:, psl], rk[:, ksl], rq[:, csl], start=True, stop=True
                        )
                        probs = ppool.tile([128, QB], dt.float32r, tag="probs")
                        nc.scalar.activation(
                            probs[:, psl],
                            sc[:, psl],
                            mybir.ActivationFunctionType.Exp,
                            scale=SCALE,
                        )
                        emit_masks(probs, kinds)
                        wprobs.append(probs)

                    is_last_wave = w0 + WAVE >= n_tiles

                    def pv_emit(
                        u=u, qb=qb, wave=wave, wprobs=wprobs,
                        w0=w0, n_tiles=n_tiles, last_wave=is_last_wave,
                    ):
                        outT_ps, den_ps = get_psums()
                        for wi, (kj, kinds, t0, t1) in enumerate(wave):
                            psl = slice(t0 * 128, t1 * 128)
                            ksl = slice(kj * 128, (kj + 1) * 128)
                            first = kj == 0
                            last = w0 + wi == n_tiles - 1
                            nc.tensor.matmul(
                                outT_ps[:, psl], v_sb[:, ksl], wprobs[wi][:, psl],
                                start=first, stop=last,
                            )
                            nc.tensor.matmul(
                                den_ps[:, psl], ones_sb[:], wprobs[wi][:, psl],
                                start=first, stop=last,
                            )
                        if last_wave:
                            # normalize now; transposes/store deferred one qb
                            rden = tpool.tile([128, QB], dt.float32, tag="rden")
                            nc.vector.reciprocal_approx_fast(rden[:], den_ps[:])
                            outN = tpool.tile([128, QB], dt.float32, tag="outN")
                            nc.vector.tensor_mul(outN[:], outT_ps[:], rden[:])

                            def tail(u=u, qb=qb, outN=outN):
                                tr = ps_sc.tile([128, QB], dt.float32, tag="sc")
                                for c in range(NQC):
                                    tsl = slice(c * 128, (c + 1) * 128)
                                    nc.tensor.transpose(
                                        tr[:, tsl], outN[:, tsl], ident_sb[:]
                                    )
                                out_sb = tpool.tile([128, QB], dt.float32, tag="out_sb")
                                nc.scalar.copy(out_sb[:], tr[:])
                                nc.sync.dma_start(
                                    out=out[u].rearrange("(n p) d -> p n d", p=128)[
                                        :, qb * NQC : (qb + 1) * NQC, :
                                    ],
                                    in_=out_sb[:].rearrange("p (n d) -> p n d", d=128),
                                )

                            if state["tail"] is not None:
                                state["tail"]()
                            state["tail"] = tail

                    state["pv"].append(pv_emit)
                    flush_pv() if len(state["pv"]) > 1 else None

            cur = load(0)
            load_consts_rest()
            for u in range(per_core):
                nxt = load(u + 1) if u + 1 < per_core else None
                for qb in range(nqb):
                    if u == 0:
                        rotary(cur, qb * QB, (qb + 1) * QB)
                    attention_qb(u, cur[3], cur[4], cur[2], qb)
                if nxt is not None:
                    rotary(nxt, 0, s)
                cur = nxt
            flush_all()
            if state["tail"] is not None:
                state["tail"]()

    nc.compile()
    return nc


def host_prep(q, k, v, cos, sin, s=S, w=LOCAL_WINDOW):
    """Build per-core input maps from full inputs."""
    b, _, h, d = q.shape
    partial = _window_partial_deltas(w)

    cexp = np.empty((128, s), dtype=np.float32)
    ssig = np.empty((128, s), dtype=np.float32)
    cexp[0::2, :] = cos.T
    cexp[1::2, :] = cos.T
    ssig[0::2, :] = -sin.T
    ssig[1::2, :] = sin.T

    pswap = np.zeros((128, 128), dtype=np.float32)
    idx = np.arange(128)
    pswap[idx, idx ^ 1] = 1.0
    ident = np.eye(128, dtype=np.float32)
    ones = np.ones((128, 128), dtype=np.float32)

    p = np.arange(128)[:, None]
    c = np.arange(128)[None, :]
    maskD = (c >= p).astype(np.float32)
    wmasks = {
        delta: ((c - p) < t).astype(np.float32) for delta, t in partial.items()
    }

    units = [(bi, hi) for bi in range(b) for hi in range(h)]
    per = len(units) // N_CORES
    in_maps = []
    for core in range(N_CORES):
        us = units[core * per : (core + 1) * per]
        qTc = np.ascontiguousarray(
            np.stack([q[bi, :, hi, :].T for bi, hi in us])
        )
        kTc = np.ascontiguousarray(
            np.stack([k[bi, :, hi, :].T for bi, hi in us])
        )
        vc = np.ascontiguousarray(np.stack([v[bi, :, hi, :] for bi, hi in us]))
        m = {
            "qT": qTc,
            "kT": kTc,
            "v": vc,
            "cexpT": cexp,
            "ssigT": ssig,
            "pswap": pswap,
            "ident": ident,
            "ones": ones,
            "maskD": maskD,
        }
        for delta, msk in wmasks.items():
            m[f"maskW{delta}"] = msk
        in_maps.append(m)
    return in_maps, units


_NC_CACHE = {}


def kernel(q, k, v, cos, sin):
    from concourse.bass_utils import run_bass_kernel_spmd

    q = np.asarray(q, dtype=np.float32)
    k = np.asarray(k, dtype=np.float32)
    v = np.asarray(v, dtype=np.float32)
    cos = np.asarray(cos, dtype=np.float32)
    sin = np.asarray(sin, dtype=np.float32)

    if "nc" not in _NC_CACHE:
        _NC_CACHE["nc"] = build_nc()
    nc = _NC_CACHE["nc"]

    in_maps, units = host_prep(q, k, v, cos, sin)
    res = run_bass_kernel_spmd(nc, in_maps, core_ids=list(range(N_CORES)))

    b, s, h, d = q.shape
    full = np.empty((b, s, h, d), dtype=np.float32)
    per = len(units) // N_CORES
    for core in range(N_CORES):
        o = res.results[core]["out"]  # [per, s, 128]
        for i, (bi, hi) in enumerate(units[core * per : (core + 1) * per]):
            full[bi, :, hi, :] = o[i]
    return full



# revision 7
# speedup vs baseline: 1.1884x; 1.1884x over previous
"""Trainium2 Bass kernel for BaseCausalWanSelfAttention (local+sink sparse attention
with interleaved rotary), SPMD across 8 NeuronCores.

Sharding: the 24 (batch, head) pairs are split 3-per-core across 8 cores; each
core runs full local+sink attention for its pairs independently (no collectives).

Design (v3):
 - all-bf16 inputs (host casts); rotary on DVE using host-supplied row-swapped
   copies of qT/kT (no PE swap matmuls, 2x DVE mode).
 - scores [k, q] layout; QK in bf16; per query-block (512 q) the k-tiles are
   processed in PAIRS sharing one 2-bank PSUM tile; one Exp ACTIVATE per pair
   over the union chunk-range (2D AP) amortizes ACT instruction overhead.
 - masks (causal diag / window edges / out-of-range chunks) applied as ONE
   combined additive -30000 matmul per masked k-tile into the score PSUM
   (ident stationary, precomputed pattern moving), so exp yields exact zeros.
 - PV and denominator matmuls in fp8e4m3 with DoubleRow perf mode (2 k-tiles
   per matmul at 0.5 cyc/col); probs written by ACT directly in fp8 with an
   exp bias of -1.5 to keep values in fp8 range (cancels in normalization).
 - query block 0 (s < 512) uses a bf16 PV/den path instead: early tokens have
   few attended keys, softmax is peaked there, and fp8 V quantization would
   pass straight through to the output.
 - output written transposed [d, S] per unit; host transposes back.
"""
import sys

sys.path.insert(0, "/opt/trn_rl_repo")

import ml_dtypes
import numpy as np

import concourse.bacc as bacc
import concourse.mybir as mybir
import concourse.tile as tile

dt = mybir.dt
BF16 = ml_dtypes.bfloat16
FP8 = ml_dtypes.float8_e4m3

B, S, H, D = 2, 3072, 12, 128
LOCAL_WINDOW = 1560
SINK = 128
N_CORES = 8
PER_CORE = (B * H) // N_CORES  # 3
QB = 512
NQC = QB // 128  # 4
NKT = S // 128  # 24
NQB = S // QB  # 6
MAXD = 13  # max k-tile delta with any valid element (w=1560)
SCALE = 1.0 / float(np.sqrt(D))
MASK_NEG = -30000.0
EXP_BIAS = -1.5
PROBS_FP8 = True
BF16_QB0 = True  # query block 0 uses bf16 probs/V (fp8 noise too visible there)

PROB_DT = dt.float8e4 if PROBS_FP8 else dt.bfloat16
DRMODE = mybir.MatmulPerfMode.DoubleRow


def kj_list(qb):
    lo = max(1, NQC * qb - MAXD)
    hi = min(NKT - 1, NQC * qb + NQC - 1)
    return [0] + list(range(lo, hi + 1))


def tile_range(qb, kj):
    """Valid chunk range [t0, t1) of k-tile kj within query block qb."""
    if kj == 0:
        return 0, NQC
    t0 = max(0, kj - NQC * qb)
    t1 = min(NQC, kj + MAXD - NQC * qb + 1)
    return t0, t1


def chunk_code(qb, kj, t):
    qi = NQC * qb + t
    if kj == 0:
        return "D" if qi == 0 else "."
    d = qi - kj
    if d < 0 or d > MAXD:
        return "F"
    if d == 0:
        return "D"
    if d == 12:
        return "W12"
    if d == 13:
        return "W13"
    return "."


def pair_info(qb):
    """[(kjA, kjB, u0, u1, [(j, patkey), ...]), ...] for query block qb."""
    kjs = kj_list(qb)
    assert len(kjs) % 2 == 0
    pairs = []
    for i in range(0, len(kjs), 2):
        a, b = kjs[i], kjs[i + 1]
        ra, rb = tile_range(qb, a), tile_range(qb, b)
        u0, u1 = min(ra[0], rb[0]), max(ra[1], rb[1])
        ms = []
        for j, kj in ((0, a), (1, b)):
            pat = tuple(chunk_code(qb, kj, t) for t in range(u0, u1))
            if any(c != "." for c in pat):
                ms.append((j, pat))
        pairs.append((a, b, u0, u1, ms))
    assert pairs[0][2] == 0 and pairs[0][3] == NQC
    return pairs


def all_patterns():
    pats = {}
    for qb in range(NQB):
        for _, _, _, _, ms in pair_info(qb):
            for _, pat in ms:
                if pat not in pats:
                    pats[pat] = f"cm{len(pats)}"
    return pats


PATTERNS = all_patterns()


def build_nc(s=S, per_core=PER_CORE):
    nc = bacc.Bacc("TRN2", target_bir_lowering=False, debug=False)

    qT = nc.declare_dram_parameter("qT", [per_core, 128, s], dt.bfloat16, isOutput=False)
    qTs = nc.declare_dram_parameter("qTs", [per_core, 128, s], dt.bfloat16, isOutput=False)
    kT = nc.declare_dram_parameter("kT", [per_core, 128, s], dt.bfloat16, isOutput=False)
    kTs = nc.declare_dram_parameter("kTs", [per_core, 128, s], dt.bfloat16, isOutput=False)
    vp = nc.declare_dram_parameter("vp", [per_core, 128, s], PROB_DT, isOutput=False)
    vhead = nc.declare_dram_parameter("vhead", [per_core, 128, QB], dt.bfloat16, isOutput=False)
    cexpT = nc.declare_dram_parameter("cexpT", [128, s], dt.bfloat16, isOutput=False)
    ssigT = nc.declare_dram_parameter("ssigT", [128, s], dt.bfloat16, isOutput=False)
    ident = nc.declare_dram_parameter("ident", [128, 128], dt.bfloat16, isOutput=False)
    ebias = nc.declare_dram_parameter("ebias", [128, 1], dt.float32, isOutput=False)
    ones2 = nc.declare_dram_parameter("ones2", [128, 256], PROB_DT, isOutput=False)
    ones2b = nc.declare_dram_parameter("ones2b", [128, 256], dt.bfloat16, isOutput=False)
    cmask_d = {
        pat: nc.declare_dram_parameter(nm, [128, 128 * len(pat)], dt.bfloat16, isOutput=False)
        for pat, nm in PATTERNS.items()
    }
    outD = nc.declare_dram_parameter("outD", [per_core, 128, s], dt.bfloat16, isOutput=True)

    with tile.TileContext(nc) as tc:
        with (
            tc.tile_pool(name="const", bufs=1) as cpool,
            tc.tile_pool(name="unit", bufs=2) as upool,
            tc.tile_pool(name="probs", bufs=6) as ppool,
            tc.tile_pool(name="probsA", bufs=2) as papool,
            tc.tile_pool(name="tail", bufs=2) as tpool,
            tc.tile_pool(name="ps_sc", bufs=2, space="PSUM") as ps_sc,
            tc.tile_pool(name="ps_out", bufs=2, space="PSUM") as ps_out,
            tc.tile_pool(name="ps_den", bufs=2, space="PSUM") as ps_den,
        ):
            cexp_sb = cpool.tile([128, s], dt.bfloat16, tag="cexp")
            ssig_sb = cpool.tile([128, s], dt.bfloat16, tag="ssig")
            ident_sb = cpool.tile([128, 128], dt.bfloat16, tag="ident")
            ebias_sb = cpool.tile([128, 1], dt.float32, tag="ebias")
            ones2_sb = cpool.tile([128, 256], PROB_DT, tag="ones2")
            ones2b_sb = cpool.tile([128, 256], dt.bfloat16, tag="ones2b")
            cmask_sb = {
                pat: cpool.tile([128, 128 * len(pat)], dt.bfloat16, tag=nm, name=nm)
                for pat, nm in PATTERNS.items()
            }

            def load_small_consts():
                nc.sync.dma_start(out=ident_sb[:], in_=ident[:])
                nc.sync.dma_start(out=ebias_sb[:], in_=ebias[:])
                nc.sync.dma_start(out=ones2_sb[:], in_=ones2[:])
                nc.sync.dma_start(out=ones2b_sb[:], in_=ones2b[:])
                for pat, m in cmask_sb.items():
                    nc.sync.dma_start(out=m[:], in_=cmask_d[pat][:])

            def alloc_unit(u):
                return {
                    nm: upool.tile(
                        [128, QB] if nm == "vh" else [128, s],
                        PROB_DT if nm == "v" else dt.bfloat16,
                        tag=nm,
                        name=f"{nm}{u}",
                    )
                    for nm in ("qraw", "qsw", "kraw", "ksw", "rq", "rk", "v", "vh")
                }

            def load_unit0(t):
                """Unit 0: interleave const and input chunks so rotary/attention
                start as early as possible."""
                for c in range(NQB):
                    sl = slice(c * 512, (c + 1) * 512)
                    nc.sync.dma_start(out=cexp_sb[:, sl], in_=cexpT[:, sl])
                    nc.sync.dma_start(out=ssig_sb[:, sl], in_=ssigT[:, sl])
                    for nm, src in (("qraw", qT), ("qsw", qTs), ("kraw", kT), ("ksw", kTs)):
                        nc.sync.dma_start(out=t[nm][:, sl], in_=src[0][:, sl])
                    if c == 0:
                        load_small_consts()
                        nc.sync.dma_start(out=t["vh"][:], in_=vhead[0][:])
                    if c == 2:
                        nc.sync.dma_start(out=t["v"][:, 0:1536], in_=vp[0][:, 0:1536])
                nc.sync.dma_start(out=t["v"][:, 1536:s], in_=vp[0][:, 1536:s])

            def load_unit(u, t):
                for c in range(2):
                    sl = slice(c * 1536, (c + 1) * 1536)
                    for nm, src in (
                        ("qraw", qT), ("qsw", qTs), ("kraw", kT), ("ksw", kTs), ("v", vp),
                    ):
                        nc.sync.dma_start(out=t[nm][:, sl], in_=src[u][:, sl])
                nc.sync.dma_start(out=t["vh"][:], in_=vhead[u][:])

            def rot_chunk(t, c):
                """Rotary for cols [c*512,(c+1)*512) of both q and k."""
                sl = slice(c * 512, (c + 1) * 512)
                for raw, sw, r in (
                    (t["qraw"], t["qsw"], t["rq"]),
                    (t["kraw"], t["ksw"], t["rk"]),
                ):
                    nc.vector.tensor_mul(r[:, sl], raw[:, sl], cexp_sb[:, sl])
                    nc.vector.tensor_mul(sw[:, sl], sw[:, sl], ssig_sb[:, sl])
                    nc.vector.tensor_add(r[:, sl], r[:, sl], sw[:, sl])

            state = {"pv": []}

            def flush_pv(n=1):
                for _ in range(min(n, len(state["pv"]))):
                    state["pv"].pop(0)()

            def attention(u, t, qb):
                pairs = pair_info(qb)
                npairs = len(pairs)
                bf_path = BF16_QB0 and qb == 0 and PROBS_FP8
                pdt = dt.bfloat16 if bf_path else PROB_DT
                pool = papool if bf_path else ppool
                v3 = t["v"].rearrange("p (n d) -> p n d", d=128)
                vh3 = t["vh"].rearrange("p (n d) -> p n d", d=128)
                qbctx = {}

                def get_acc():
                    if "outT" not in qbctx:
                        qbctx["outT"] = ps_out.tile(
                            [128, QB], dt.float32, tag="outT", name=f"outT{u}_{qb}"
                        )
                        qbctx["den"] = ps_den.tile(
                            [128, QB], dt.float32, tag="den", name=f"den{u}_{qb}"
                        )
                    return qbctx["outT"], qbctx["den"]

                for g, (kjA, kjB, u0, u1, ms) in enumerate(pairs):
                    sc = ps_sc.tile([128, 2 * QB], dt.float32, tag="sc")
                    masked = {0: False, 1: False}
                    for j, _ in ms:
                        masked[j] = True
                    for j, kj in ((0, kjA), (1, kjB)):
                        csl = slice(qb * QB + u0 * 128, qb * QB + u1 * 128)
                        osl = slice(j * QB + u0 * 128, j * QB + u1 * 128)
                        nc.tensor.matmul(
                            sc[:, osl], t["rk"][:, kj * 128 : (kj + 1) * 128],
                            t["rq"][:, csl], start=True, stop=not masked[j],
                        )
                    for j, pat in ms:
                        msl = slice(j * QB + u0 * 128, j * QB + u1 * 128)
                        nc.tensor.matmul(
                            sc[:, msl], ident_sb[:], cmask_sb[pat][:],
                            start=False, stop=True, skip_group_check=True,
                        )
                    probs = pool.tile(
                        [128, 2 * QB], pdt, tag="probs", name=f"pr{u}_{qb}_{g}"
                    )
                    sc3 = sc[:].rearrange("p (j c) -> p j c", j=2)[:, :, u0 * 128 : u1 * 128]
                    pr3 = probs[:].rearrange("p (j c) -> p j c", j=2)[:, :, u0 * 128 : u1 * 128]
                    nc.scalar.activation(
                        pr3, sc3, mybir.ActivationFunctionType.Exp,
                        scale=SCALE, bias=ebias_sb[:],
                    )

                    def pv_emit(
                        g=g, kjA=kjA, kjB=kjB, u0=u0, u1=u1,
                        probs=probs, last=(g == npairs - 1), bf_path=bf_path,
                    ):
                        outT, den = get_acc()
                        rhs = probs[:].rearrange("p (j c) -> p j c", j=2)[
                            :, :, u0 * 128 : u1 * 128
                        ]
                        osl = slice(u0 * 128, u1 * 128)
                        if not bf_path:
                            dstep = kjB - kjA
                            vpair = v3[:, kjA : kjB + 1 : dstep, :]
                            o3 = ones2_sb[:].rearrange("p (j d) -> p j d", j=2)
                            nc.tensor.matmul(
                                outT[:, osl], vpair, rhs,
                                start=(g == 0), stop=last, perf_mode=DRMODE,
                            )
                            nc.tensor.matmul(
                                den[:, osl], o3, rhs,
                                start=(g == 0), stop=last, perf_mode=DRMODE,
                            )
                        else:
                            o3 = ones2b_sb[:].rearrange("p (j d) -> p j d", j=2)
                            for j, kj in ((0, kjA), (1, kjB)):
                                nc.tensor.matmul(
                                    outT[:, osl], vh3[:, kj, :], rhs[:, j],
                                    start=(g == 0 and j == 0), stop=(last and j == 1),
                                )
                                nc.tensor.matmul(
                                    den[:, osl], o3[:, j], rhs[:, j],
                                    start=(g == 0 and j == 0), stop=(last and j == 1),
                                )

                    state["pv"].append(pv_emit)
                    if g >= 1:
                        flush_pv()
                flush_pv(10**9)

                outT, den = get_acc()
                rden = tpool.tile([128, QB], dt.float32, tag="rden")
                nc.vector.reciprocal_approx_fast(rden[:], den[:])
                outN = tpool.tile([128, QB], dt.bfloat16, tag="outN")
                nc.vector.tensor_mul(outN[:], outT[:], rden[:])
                nc.sync.dma_start(
                    out=outD[u][:, qb * QB : (qb + 1) * QB], in_=outN[:]
                )

            cur = alloc_unit(0)
            load_unit0(cur)
            nxt = None
            for u in range(per_core):
                for qb in range(NQB):
                    if u == 0:
                        rot_chunk(cur, qb)
                    attention(u, cur, qb)
                    if u == 0 and qb == 0 and per_core > 1:
                        nxt = alloc_unit(1)
                        load_unit(1, nxt)
                    if nxt is not None and u < per_core - 1:
                        rot_chunk(nxt, qb)
                if u + 1 < per_core:
                    cur = nxt
                    if u + 2 < per_core:
                        nxt = alloc_unit(u + 2)
                        load_unit(u + 2, nxt)
                    else:
                        nxt = None

    nc.compile()
    return nc


def host_prep(q, k, v, cos, sin, s=S):
    b, _, h, d = q.shape

    cexp = np.empty((128, s), dtype=np.float32)
    ssig = np.empty((128, s), dtype=np.float32)
    cexp[0::2, :] = cos.T
    cexp[1::2, :] = cos.T
    ssig[0::2, :] = -sin.T
    ssig[1::2, :] = sin.T

    ident = np.eye(128, dtype=np.float32)
    ones2 = np.ones((128, 256), dtype=np.float32)

    p = np.arange(128)[:, None]
    c = np.arange(128)[None, :]
    base = {
        ".": np.zeros((128, 128), dtype=np.float32),
        "D": np.where(c >= p, 0.0, MASK_NEG).astype(np.float32),
        "W12": np.where((c - p) < 24, 0.0, MASK_NEG).astype(np.float32),
        "W13": np.where((c - p) < -104, 0.0, MASK_NEG).astype(np.float32),
        "F": np.full((128, 128), MASK_NEG, dtype=np.float32),
    }
    cmasks = {
        nm: np.hstack([base[cc] for cc in pat]) for pat, nm in PATTERNS.items()
    }

    perm = np.arange(128) ^ 1
    units = [(bi, hi) for bi in range(b) for hi in range(h)]
    per = len(units) // N_CORES
    prob_np = FP8 if PROBS_FP8 else BF16
    in_maps = []
    for core in range(N_CORES):
        us = units[core * per : (core + 1) * per]
        qTc = np.stack([np.ascontiguousarray(q[bi, :, hi, :].T) for bi, hi in us])
        kTc = np.stack([np.ascontiguousarray(k[bi, :, hi, :].T) for bi, hi in us])
        # v rearranged to [128, n*128] with vr[p, n*128+d] = v[n*128+p, d]
        vc = np.stack(
            [
                np.ascontiguousarray(
                    v[bi, :, hi, :]
                    .reshape(s // 128, 128, 128)
                    .transpose(1, 0, 2)
                    .reshape(128, s)
                )
                for bi, hi in us
            ]
        )
        m = {
            "qT": qTc.astype(BF16),
            "qTs": qTc[:, perm, :].astype(BF16),
            "kT": kTc.astype(BF16),
            "kTs": kTc[:, perm, :].astype(BF16),
            "vp": vc.astype(prob_np),
            "vhead": vc[:, :, 0:QB].astype(BF16),
            "cexpT": cexp.astype(BF16),
            "ssigT": ssig.astype(BF16),
            "ebias": np.full((128, 1), EXP_BIAS, dtype=np.float32),
            "ident": ident.astype(BF16),
            "ones2": ones2.astype(prob_np),
            "ones2b": ones2.astype(BF16),
        }
        for nm, msk in cmasks.items():
            m[nm] = msk.astype(BF16)
        in_maps.append(m)
    return in_maps, units


_NC_CACHE = {}


def kernel(q, k, v, cos, sin):
    from concourse.bass_utils import run_bass_kernel_spmd

    q = np.asarray(q, dtype=np.float32)
    k = np.asarray(k, dtype=np.float32)
    v = np.asarray(v, dtype=np.float32)
    cos = np.asarray(cos, dtype=np.float32)
    sin = np.asarray(sin, dtype=np.float32)

    if "nc" not in _NC_CACHE:
        _NC_CACHE["nc"] = build_nc()
    nc = _NC_CACHE["nc"]

    in_maps, units = host_prep(q, k, v, cos, sin)
    res = run_bass_kernel_spmd(nc, in_maps, core_ids=list(range(N_CORES)))

    b, s, h, d = q.shape
    full = np.empty((b, s, h, d), dtype=np.float32)
    per = len(units) // N_CORES
    for core in range(N_CORES):
        o = res.results[core]["outD"]  # [per, 128, s] bf16, transposed layout
        for i, (bi, hi) in enumerate(units[core * per : (core + 1) * per]):
            full[bi, :, hi, :] = o[i].T.astype(np.float32)
    return full


# revision 9
# speedup vs baseline: 1.4071x; 1.1840x over previous
"""Trainium2 Bass kernel for BaseCausalWanSelfAttention (local+sink sparse attention
with interleaved rotary), SPMD across 8 NeuronCores.

Sharding: the 24 (batch, head) pairs are split 3-per-core across 8 cores; each
core runs full local+sink attention for its pairs independently (no collectives).

Design (v3):
 - all-bf16 inputs (host casts); rotary on DVE using host-supplied row-swapped
   copies of qT/kT (no PE swap matmuls, 2x DVE mode).
 - scores [k, q] layout; QK in bf16; per query-block (512 q) the k-tiles are
   processed in PAIRS sharing one 2-bank PSUM tile; one Exp ACTIVATE per pair
   over the union chunk-range (2D AP) amortizes ACT instruction overhead.
 - masks (causal diag / window edges / out-of-range chunks) applied as ONE
   combined additive -30000 matmul per masked k-tile into the score PSUM
   (ident stationary, precomputed pattern moving), so exp yields exact zeros.
 - PV and denominator matmuls in fp8e4m3 with DoubleRow perf mode (2 k-tiles
   per matmul at 0.5 cyc/col); probs written by ACT directly in fp8 with an
   exp bias of -1.5 to keep values in fp8 range (cancels in normalization).
 - query block 0 (s < 512) uses a bf16 PV/den path instead: early tokens have
   few attended keys, softmax is peaked there, and fp8 V quantization would
   pass straight through to the output.
 - output written transposed [d, S] per unit; host transposes back.
"""
import sys

sys.path.insert(0, "/opt/trn_rl_repo")

import ml_dtypes
import numpy as np

import concourse.bacc as bacc
import concourse.mybir as mybir
import concourse.tile as tile

dt = mybir.dt
BF16 = ml_dtypes.bfloat16
FP8 = ml_dtypes.float8_e4m3

B, S, H, D = 2, 3072, 12, 128
LOCAL_WINDOW = 1560
SINK = 128
N_CORES = 8
PER_CORE = (B * H) // N_CORES  # 3
QB = 512
NQC = QB // 128  # 4
NKT = S // 128  # 24
NQB = S // QB  # 6
MAXD = 13  # max k-tile delta with any valid element (w=1560)
SCALE = 1.0 / float(np.sqrt(D))
MASK_NEG = -30000.0
EXP_BIAS = -1.5
PROBS_FP8 = True
BF16_QB0 = True  # query block 0 uses bf16 probs/V (fp8 noise too visible there)

PROB_DT = dt.float8e4 if PROBS_FP8 else dt.bfloat16
DRMODE = mybir.MatmulPerfMode.DoubleRow


def kj_list(qb):
    lo = max(1, NQC * qb - MAXD)
    hi = min(NKT - 1, NQC * qb + NQC - 1)
    return [0] + list(range(lo, hi + 1))


def tile_range(qb, kj):
    """Valid chunk range [t0, t1) of k-tile kj within query block qb."""
    if kj == 0:
        return 0, NQC
    t0 = max(0, kj - NQC * qb)
    t1 = min(NQC, kj + MAXD - NQC * qb + 1)
    return t0, t1


def chunk_code(qb, kj, t):
    qi = NQC * qb + t
    if kj == 0:
        return "D" if qi == 0 else "."
    d = qi - kj
    if d < 0 or d > MAXD:
        return "F"
    if d == 0:
        return "D"
    if d == 12:
        return "W12"
    if d == 13:
        return "W13"
    return "."


def pair_info(qb):
    """[(kjA, kjB, u0, u1, [(j, patkey), ...]), ...] for query block qb."""
    kjs = kj_list(qb)
    assert len(kjs) % 2 == 0
    pairs = []
    for i in range(0, len(kjs), 2):
        a, b = kjs[i], kjs[i + 1]
        ra, rb = tile_range(qb, a), tile_range(qb, b)
        u0, u1 = min(ra[0], rb[0]), max(ra[1], rb[1])
        ms = []
        for j, kj in ((0, a), (1, b)):
            pat = tuple(chunk_code(qb, kj, t) for t in range(u0, u1))
            if any(c != "." for c in pat):
                ms.append((j, pat))
        pairs.append((a, b, u0, u1, ms))
    assert pairs[0][2] == 0 and pairs[0][3] == NQC
    return pairs


def all_patterns():
    pats = {}
    for qb in range(NQB):
        for _, _, _, _, ms in pair_info(qb):
            for _, pat in ms:
                if pat not in pats:
                    pats[pat] = f"cm{len(pats)}"
    return pats


PATTERNS = all_patterns()


def build_nc(s=S, per_core=PER_CORE):
    nc = bacc.Bacc("TRN2", target_bir_lowering=False, debug=False)

    qT = nc.declare_dram_parameter("qT", [per_core, 128, s], dt.bfloat16, isOutput=False)
    qTs = nc.declare_dram_parameter("qTs", [per_core, 128, s], dt.bfloat16, isOutput=False)
    kT = nc.declare_dram_parameter("kT", [per_core, 128, s], dt.bfloat16, isOutput=False)
    kTs = nc.declare_dram_parameter("kTs", [per_core, 128, s], dt.bfloat16, isOutput=False)
    vp = nc.declare_dram_parameter("vp", [per_core, 128, s], PROB_DT, isOutput=False)
    vhead = nc.declare_dram_parameter("vhead", [per_core, 128, QB], dt.bfloat16, isOutput=False)
    cexpT = nc.declare_dram_parameter("cexpT", [128, s], dt.bfloat16, isOutput=False)
    ssigT = nc.declare_dram_parameter("ssigT", [128, s], dt.bfloat16, isOutput=False)
    ident = nc.declare_dram_parameter("ident", [128, 128], dt.bfloat16, isOutput=False)
    ebias = nc.declare_dram_parameter("ebias", [128, 1], dt.float32, isOutput=False)
    ones2 = nc.declare_dram_parameter("ones2", [128, 256], PROB_DT, isOutput=False)
    ones2b = nc.declare_dram_parameter("ones2b", [128, 256], dt.bfloat16, isOutput=False)
    cmask_d = {
        pat: nc.declare_dram_parameter(nm, [128, 128 * len(pat)], dt.bfloat16, isOutput=False)
        for pat, nm in PATTERNS.items()
    }
    outD = nc.declare_dram_parameter("outD", [per_core, 128, s], dt.bfloat16, isOutput=True)

    with tile.TileContext(nc) as tc:
        with (
            tc.tile_pool(name="const", bufs=1) as cpool,
            tc.tile_pool(name="unit", bufs=2) as upool,
            tc.tile_pool(name="probs", bufs=6) as ppool,
            tc.tile_pool(name="probsA", bufs=2) as papool,
            tc.tile_pool(name="tail", bufs=2) as tpool,
            tc.tile_pool(name="ps_sc", bufs=3, space="PSUM") as ps_sc,
            tc.tile_pool(name="ps_out", bufs=1, space="PSUM") as ps_out,
            tc.tile_pool(name="ps_den", bufs=1, space="PSUM") as ps_den,
        ):
            cexp_sb = cpool.tile([128, s], dt.bfloat16, tag="cexp")
            ssig_sb = cpool.tile([128, s], dt.bfloat16, tag="ssig")
            ident_sb = cpool.tile([128, 128], dt.bfloat16, tag="ident")
            ebias_sb = cpool.tile([128, 1], dt.float32, tag="ebias")
            ones2_sb = cpool.tile([128, 256], PROB_DT, tag="ones2")
            ones2b_sb = cpool.tile([128, 256], dt.bfloat16, tag="ones2b")
            cmask_sb = {
                pat: cpool.tile([128, 128 * len(pat)], dt.bfloat16, tag=nm, name=nm)
                for pat, nm in PATTERNS.items()
            }

            def load_small_consts():
                nc.gpsimd.dma_start(out=ident_sb[:], in_=ident[:])
                nc.gpsimd.dma_start(out=ebias_sb[:], in_=ebias[:])
                nc.gpsimd.dma_start(out=ones2_sb[:], in_=ones2[:])
                nc.gpsimd.dma_start(out=ones2b_sb[:], in_=ones2b[:])
                for pat, m in cmask_sb.items():
                    nc.gpsimd.dma_start(out=m[:], in_=cmask_d[pat][:])

            def alloc_unit(u):
                return {
                    nm: upool.tile(
                        [128, QB] if nm == "vh" else [128, s],
                        PROB_DT if nm == "v" else dt.bfloat16,
                        tag=nm,
                        name=f"{nm}{u}",
                    )
                    for nm in ("qraw", "qsw", "kraw", "ksw", "rq", "rk", "v", "vh")
                }

            def load_unit0(t):
                """Unit 0: interleave const and input chunks so rotary/attention
                start as early as possible."""
                for c in range(NQB):
                    sl = slice(c * 512, (c + 1) * 512)
                    nc.sync.dma_start(out=cexp_sb[:, sl], in_=cexpT[:, sl])
                    nc.sync.dma_start(out=ssig_sb[:, sl], in_=ssigT[:, sl])
                    for nm, src in (("qraw", qT), ("qsw", qTs), ("kraw", kT), ("ksw", kTs)):
                        nc.sync.dma_start(out=t[nm][:, sl], in_=src[0][:, sl])
                    if c == 0:
                        load_small_consts()
                        nc.gpsimd.dma_start(out=t["vh"][:], in_=vhead[0][:])
                    if c == 2:
                        nc.gpsimd.dma_start(out=t["v"][:, 0:1536], in_=vp[0][:, 0:1536])
                nc.gpsimd.dma_start(out=t["v"][:, 1536:s], in_=vp[0][:, 1536:s])

            def load_unit(u, t):
                for nm, srcp in (
                    ("qraw", qT), ("qsw", qTs), ("kraw", kT), ("ksw", kTs),
                ):
                    nc.sync.dma_start(out=t[nm][:], in_=srcp[u][:])
                nc.gpsimd.dma_start(out=t["v"][:], in_=vp[u][:])
                nc.gpsimd.dma_start(out=t["vh"][:], in_=vhead[u][:])

            def rot_chunk(t, c):
                """Rotary for cols [c*512,(c+1)*512) of both q and k."""
                sl = slice(c * 512, (c + 1) * 512)
                for raw, sw, r in (
                    (t["qraw"], t["qsw"], t["rq"]),
                    (t["kraw"], t["ksw"], t["rk"]),
                ):
                    nc.vector.tensor_mul(r[:, sl], raw[:, sl], cexp_sb[:, sl])
                    nc.vector.tensor_mul(sw[:, sl], sw[:, sl], ssig_sb[:, sl])
                    nc.vector.tensor_add(r[:, sl], r[:, sl], sw[:, sl])

            state = {"pv": []}

            def flush_pv(n=1):
                for _ in range(min(n, len(state["pv"]))):
                    state["pv"].pop(0)()

            def attention(u, t, qb):
                pairs = pair_info(qb)
                npairs = len(pairs)
                bf_path = BF16_QB0 and qb == 0 and PROBS_FP8
                pdt = dt.bfloat16 if bf_path else PROB_DT
                pool = papool if bf_path else ppool
                v3 = t["v"].rearrange("p (n d) -> p n d", d=128)
                vh3 = t["vh"].rearrange("p (n d) -> p n d", d=128)
                qbctx = {}

                def get_acc():
                    if "outT" not in qbctx:
                        qbctx["outT"] = ps_out.tile(
                            [128, QB], dt.float32, tag="outT", name=f"outT{u}_{qb}"
                        )
                        qbctx["den"] = ps_den.tile(
                            [128, QB], dt.float32, tag="den", name=f"den{u}_{qb}"
                        )
                    return qbctx["outT"], qbctx["den"]

                for g, (kjA, kjB, u0, u1, ms) in enumerate(pairs):
                    sc = ps_sc.tile([128, 2 * QB], dt.float32, tag="sc")
                    masked = {0: False, 1: False}
                    for j, _ in ms:
                        masked[0] |= j in (0, 2)
                        masked[1] |= j in (1, 2)
                    for j, kj in ((0, kjA), (1, kjB)):
                        csl = slice(qb * QB + u0 * 128, qb * QB + u1 * 128)
                        osl = slice(j * QB + u0 * 128, j * QB + u1 * 128)
                        nc.tensor.matmul(
                            sc[:, osl], t["rk"][:, kj * 128 : (kj + 1) * 128],
                            t["rq"][:, csl], start=True, stop=not masked[j],
                        )
                    for j, pat in ms:
                        if j == 2:
                            msl = slice(0, 2 * QB)
                        else:
                            msl = slice(j * QB + u0 * 128, j * QB + u1 * 128)
                        nc.tensor.matmul(
                            sc[:, msl], ident_sb[:], cmask_sb[pat][:],
                            start=False, stop=True, skip_group_check=True,
                        )
                    probs = pool.tile(
                        [128, 2 * QB], pdt, tag="probs", name=f"pr{u}_{qb}_{g}"
                    )
                    sc3 = sc[:].rearrange("p (j c) -> p j c", j=2)[:, :, u0 * 128 : u1 * 128]
                    pr3 = probs[:].rearrange("p (j c) -> p j c", j=2)[:, :, u0 * 128 : u1 * 128]
                    nc.scalar.activation(
                        pr3, sc3, mybir.ActivationFunctionType.Exp,
                        scale=SCALE, bias=ebias_sb[:],
                    )

                    def pv_emit(
                        g=g, kjA=kjA, kjB=kjB, u0=u0, u1=u1,
                        probs=probs, last=(g == npairs - 1), bf_path=bf_path,
                    ):
                        outT, den = get_acc()
                        rhs = probs[:].rearrange("p (j c) -> p j c", j=2)[
                            :, :, u0 * 128 : u1 * 128
                        ]
                        osl = slice(u0 * 128, u1 * 128)
                        if not bf_path:
                            dstep = kjB - kjA
                            vpair = v3[:, kjA : kjB + 1 : dstep, :]
                            o3 = ones2_sb[:].rearrange("p (j d) -> p j d", j=2)
                            nc.tensor.matmul(
                                outT[:, osl], vpair, rhs,
                                start=(g == 0), stop=last, perf_mode=DRMODE,
                            )
                            nc.tensor.matmul(
                                den[:, osl], o3, rhs,
                                start=(g == 0), stop=last, perf_mode=DRMODE,
                            )
                        else:
                            o3 = ones2b_sb[:].rearrange("p (j d) -> p j d", j=2)
                            for j, kj in ((0, kjA), (1, kjB)):
                                nc.tensor.matmul(
                                    outT[:, osl], vh3[:, kj, :], rhs[:, j],
                                    start=(g == 0 and j == 0), stop=(last and j == 1),
                                )
                                nc.tensor.matmul(
                                    den[:, osl], o3[:, j], rhs[:, j],
                                    start=(g == 0 and j == 0), stop=(last and j == 1),
                                )

                    state["pv"].append(pv_emit)
                    if g >= 2:
                        flush_pv()
                flush_pv(10**9)

                outT, den = get_acc()
                rden = tpool.tile([128, QB], dt.float32, tag="rden")
                nc.vector.reciprocal_approx_fast(rden[:], den[:])
                outN = tpool.tile([128, QB], dt.bfloat16, tag="outN")
                nc.vector.tensor_mul(outN[:], outT[:], rden[:])
                nc.gpsimd.dma_start(
                    out=outD[u][:, qb * QB : (qb + 1) * QB], in_=outN[:]
                )

            cur = alloc_unit(0)
            load_unit0(cur)
            nxt = None
            for u in range(per_core):
                for qb in range(NQB):
                    if u == 0:
                        rot_chunk(cur, qb)
                    attention(u, cur, qb)
                    if u == 0 and qb == 0 and per_core > 1:
                        nxt = alloc_unit(1)
                        load_unit(1, nxt)
                    if nxt is not None and u < per_core - 1:
                        rot_chunk(nxt, qb)
                if u + 1 < per_core:
                    cur = nxt
                    if u + 2 < per_core:
                        nxt = alloc_unit(u + 2)
                        load_unit(u + 2, nxt)
                    else:
                        nxt = None

    nc.compile()
    return nc


def host_prep(q, k, v, cos, sin, s=S):
    b, _, h, d = q.shape

    cexp = np.empty((128, s), dtype=np.float32)
    ssig = np.empty((128, s), dtype=np.float32)
    cexp[0::2, :] = cos.T
    cexp[1::2, :] = cos.T
    ssig[0::2, :] = -sin.T
    ssig[1::2, :] = sin.T

    ident = np.eye(128, dtype=np.float32)
    ones2 = np.ones((128, 256), dtype=np.float32)

    p = np.arange(128)[:, None]
    c = np.arange(128)[None, :]
    base = {
        ".": np.zeros((128, 128), dtype=np.float32),
        "D": np.where(c >= p, 0.0, MASK_NEG).astype(np.float32),
        "W12": np.where((c - p) < 24, 0.0, MASK_NEG).astype(np.float32),
        "W13": np.where((c - p) < -104, 0.0, MASK_NEG).astype(np.float32),
        "F": np.full((128, 128), MASK_NEG, dtype=np.float32),
    }
    cmasks = {
        nm: np.hstack([base[cc] for cc in pat]) for pat, nm in PATTERNS.items()
    }

    perm = np.arange(128) ^ 1
    units = [(bi, hi) for bi in range(b) for hi in range(h)]
    per = len(units) // N_CORES
    prob_np = FP8 if PROBS_FP8 else BF16
    in_maps = []
    for core in range(N_CORES):
        us = units[core * per : (core + 1) * per]
        qTc = np.stack([np.ascontiguousarray(q[bi, :, hi, :].T) for bi, hi in us])
        kTc = np.stack([np.ascontiguousarray(k[bi, :, hi, :].T) for bi, hi in us])
        # v rearranged to [128, n*128] with vr[p, n*128+d] = v[n*128+p, d]
        vc = np.stack(
            [
                np.ascontiguousarray(
                    v[bi, :, hi, :]
                    .reshape(s // 128, 128, 128)
                    .transpose(1, 0, 2)
                    .reshape(128, s)
                )
                for bi, hi in us
            ]
        )
        m = {
            "qT": qTc.astype(BF16),
            "qTs": qTc[:, perm, :].astype(BF16),
            "kT": kTc.astype(BF16),
            "kTs": kTc[:, perm, :].astype(BF16),
            "vp": vc.astype(prob_np),
            "vhead": vc[:, :, 0:QB].astype(BF16),
            "cexpT": cexp.astype(BF16),
            "ssigT": ssig.astype(BF16),
            "ebias": np.full((128, 1), EXP_BIAS, dtype=np.float32),
            "ident": ident.astype(BF16),
            "ones2": ones2.astype(prob_np),
            "ones2b": ones2.astype(BF16),
        }
        for nm, msk in cmasks.items():
            m[nm] = msk.astype(BF16)
        in_maps.append(m)
    return in_maps, units


_NC_CACHE = {}


def kernel(q, k, v, cos, sin):
    from concourse.bass_utils import run_bass_kernel_spmd

    q = np.asarray(q, dtype=np.float32)
    k = np.asarray(k, dtype=np.float32)
    v = np.asarray(v, dtype=np.float32)
    cos = np.asarray(cos, dtype=np.float32)
    sin = np.asarray(sin, dtype=np.float32)

    if "nc" not in _NC_CACHE:
        _NC_CACHE["nc"] = build_nc()
    nc = _NC_CACHE["nc"]

    in_maps, units = host_prep(q, k, v, cos, sin)
    res = run_bass_kernel_spmd(nc, in_maps, core_ids=list(range(N_CORES)))

    b, s, h, d = q.shape
    full = np.empty((b, s, h, d), dtype=np.float32)
    per = len(units) // N_CORES
    for core in range(N_CORES):
        o = res.results[core]["outD"]  # [per, 128, s] bf16, transposed layout
        for i, (bi, hi) in enumerate(units[core * per : (core + 1) * per]):
            full[bi, :, hi, :] = o[i].T.astype(np.float32)
    return full
